# revision 25
# baseline (speedup 1.0000x reference)
"""CapsuleLayer (B=32, J=32, I=2048, T=16, D=16, 3 routing iters) on 8 TRN2 cores.

Strategy: shard input-capsule axis I across the 8 cores (I_loc = 256).
W reads at the HBM roofline (bf16, 4.2 MB/core); all routing state except
the tiny s[b,j,t] (64 KB, AllReduce x3) is core-local.

Per core:
  - u_hat on TensorE: K=(i_sub 4, d 16)=64, M=(i_sub 4, b 32)=128,
    N=(t,j)=512, host-prepacked block-diagonal x weights (bf16) and W
    streamed as bf16 rhs.  W host layout puts free=(t,j) so the PSUM->SBUF
    u copy is a contiguous cast (2x mode), alternating ACT/DVE.
  - s0 partial fused into the same W pass (xplain lhsT), single PSUM
    accumulation group across all tiles (skip_group_check).
  - routing: q = u*v on DVE (bf16 2x) into (t,g,j) layout; the t-sum is
    done on TensorE (identity-weight restream, 16 accumulating matmuls
    into one PSUM bank per chunk); softmax exp on ACT with accum_out
    giving z for free; p2 = u*c on DVE; i-sum via ones-delta restream.
  - squash uses only Exp/Ln ACT functions (rsqrt via exp(0.5*ln)), so a
    single ACT table set (natural_log_exp) is loaded once - no set
    thrashing on the critical path.
  - a dummy AllReduce at t=0 warms up the collective mesh so the first
    real AllReduce only pays steady-state latency.
"""

import functools
import os
import sys

import numpy as np

sys.path.insert(0, "/opt/trn_rl_repo")

import ml_dtypes  # noqa: E402

import concourse.bass as bass  # noqa: E402
import concourse.bacc as bacc  # noqa: E402
import concourse.mybir as mybir  # noqa: E402
import concourse.tile as tile  # noqa: E402

F32 = mybir.dt.float32
BF16 = mybir.dt.bfloat16

# Make the ACT table-load pass resolve Exp and Ln to the one set that has
# both (natural_log_exp_and_others) instead of greedily alternating between
# exp_and_others and natural_log (a ~1.3us table load per switch, on the
# critical path).  Set indices are preserved; we only hide Exp/Ln from the
# other sets so the chooser can't pick them.
_orig_get_act_tables = bacc.get_activation_tables


def _patched_get_act_tables(arch):
    tables = _orig_get_act_tables(arch)
    combined = None
    for name, fns in tables.items():
        if (mybir.ActivationFunctionType.Exp in fns
                and mybir.ActivationFunctionType.Ln in fns):
            combined = name
            break
    if combined is not None:
        for name, fns in tables.items():
            if name != combined:
                fns.discard(mybir.ActivationFunctionType.Exp)
                fns.discard(mybir.ActivationFunctionType.Ln)
    return tables


bacc.get_activation_tables = _patched_get_act_tables

NCORES = 8
B, J, I, T, D = 32, 32, 2048, 16, 16
ILOC = I // NCORES          # 256
G = ILOC // 4               # 64 i-groups of 4
EPS = 1e-9

NWT = G // 2                # 32 w dram tiles, each [128, 512] covers 2 g
WCH = 8                     # w tiles per DMA chunk
NCHD = NWT // WCH           # 4 dma chunks
NCH = 8                     # routing chunks (8 g per chunk)

ExpF = mybir.ActivationFunctionType.Exp
LnF = mybir.ActivationFunctionType.Ln


def _build_program(single=False):
    nc = bacc.Bacc(
        "TRN2",
        target_bir_lowering=False,
        debug=False,
        enable_asserts=False,
        num_devices=1 if single else NCORES,
    )

    wt_d = nc.dram_tensor("wt", [NCHD, 128, WCH * 512], BF16, kind="ExternalInput")
    xd_d = nc.dram_tensor("xd", [128, NWT * 128], BF16, kind="ExternalInput")
    xp_d = nc.dram_tensor("xplain", [128, NWT * 32], BF16, kind="ExternalInput")
    id_d = nc.dram_tensor("ident", [128, 128], BF16, kind="ExternalInput")
    warm_d = nc.dram_tensor("warm", [1, 8], F32, kind="ExternalInput")
    ones_d = nc.dram_tensor("onesdb", [128, 32], BF16, kind="ExternalInput")
    repl_d = nc.dram_tensor("repl", [32, 128], F32, kind="ExternalInput")
    out_d = nc.dram_tensor("outv", [32, 512], F32, kind="ExternalOutput")

    with tile.TileContext(nc) as tc:
        _capsule(
            tc, wt_d.ap(), xd_d.ap(), xp_d.ap(), id_d.ap(), ones_d.ap(),
            repl_d.ap(), out_d.ap(), warm_d.ap(), single=single,
        )
    nc.compile()
    return nc


def _capsule(tc, wt, xd, xpd, identd, ones_dram, repl_dram, outv, warm_dram,
             single=False):
    nc = tc.nc
    from contextlib import ExitStack

    ctx = ExitStack()
    with ctx:
        up = ctx.enter_context(tc.tile_pool(name="u", bufs=1))
        wp = ctx.enter_context(tc.tile_pool(name="w", bufs=2))
        xp = ctx.enter_context(tc.tile_pool(name="x", bufs=1))
        cp = ctx.enter_context(tc.tile_pool(name="consts", bufs=1))
        qp = ctx.enter_context(tc.tile_pool(name="q", bufs=3))
        pp = ctx.enter_context(tc.tile_pool(name="p2", bufs=2))
        ep = ctx.enter_context(tc.tile_pool(name="soft", bufs=1))
        sp = ctx.enter_context(tc.tile_pool(name="small", bufs=2))
        vp = ctx.enter_context(tc.tile_pool(name="vexp", bufs=2))
        pup = ctx.enter_context(tc.tile_pool(name="upsum", bufs=2, space="PSUM"))
        ps0 = ctx.enter_context(tc.tile_pool(name="s0psum", bufs=1, space="PSUM"))
        dpp = ctx.enter_context(tc.tile_pool(name="dpsum", bufs=2, space="PSUM"))
        spp = ctx.enter_context(tc.tile_pool(name="spsum", bufs=1, space="PSUM"))
        prp = ctx.enter_context(tc.tile_pool(name="rpsum", bufs=1, space="PSUM"))
        scp = ctx.enter_context(tc.tile_pool(name="scpsum", bufs=1, space="PSUM"))
        dp = ctx.enter_context(tc.tile_pool(name="dram", bufs=8, space="DRAM"))

        # ---- collective warm-up: a dummy AllReduce rendezvous absorbs the
        # per-core ncfw boot/setup variance while phase A's DMAs run.
        if not single:
            wcc_in = dp.tile([1, 8], F32, tag="wccin", name="wcc_in")
            wcc = dp.tile([1, 8], F32, tag="wcc", name="wcc_out")
            nc.sync.dma_start(wcc_in[:, :], warm_dram)
            nc.gpsimd.collective_compute(
                "AllReduce",
                mybir.AluOpType.add,
                replica_groups=[list(range(NCORES))],
                ins=[wcc_in[:, :].opt()],
                outs=[wcc[:, :].opt()],
            )

        # ---- ACT table warm-up: Exp+Ln only => single natural_log_exp set
        wact = cp.tile([1, 8], F32, name="warm_act")
        wsb = cp.tile([1, 8], F32, name="warm_sb")
        nc.vector.memset(wsb[:, :], 1.0)
        nc.scalar.activation(wact[:, :], wsb[:, :], ExpF)
        nc.scalar.activation(wact[:, :], wsb[:, :], LnF)
        epsb = cp.tile([32, 1], F32, name="eps_bias")
        nc.vector.memset(epsb[:, :], EPS)

        # ---- persistent tiles
        u = up.tile([128, G * 512], BF16)          # [(i_sub,b), (g,t,j)]
        xall = xp.tile([128, NWT * 128], BF16)     # block-diag x lhsT
        xpl = xp.tile([128, NWT * 32], BF16)       # plain x lhsT
        ident = cp.tile([128, 128], BF16)          # identity for t-restream
        ones = cp.tile([128, 32], BF16)            # b-delta for i-restream
        repl = cp.tile([32, 128], F32)             # v replication matrix

        nc.sync.dma_start(xall[:, :], xd)
        nc.sync.dma_start(xpl[:, :], xpd)
        nc.sync.dma_start(ident[:, :], identd)
        nc.sync.dma_start(ones[:, :], ones_dram)
        nc.sync.dma_start(repl[:, :], repl_dram)

        # ---- phase A: u_hat + fused s0 partial
        cp_engines = [nc.scalar, nc.vector]
        s0p = ps0.tile([32, 512], F32, tag="s0")
        ncopy = 0
        for c in range(NCHD):
            wch = wp.tile([128, WCH * 512], BF16, tag="w")
            nc.sync.dma_start(wch[:, :], wt[c])
            for qi in range(WCH):
                p = c * WCH + qi
                for gl in range(2):
                    g = 2 * p + gl
                    ups = pup.tile([128, 512], F32)
                    nc.tensor.matmul(
                        ups[:, :],
                        lhsT=xall[gl * 64 : (gl + 1) * 64, p * 128 : (p + 1) * 128],
                        rhs=wch[gl * 64 : (gl + 1) * 64, qi * 512 : (qi + 1) * 512],
                        start=True,
                        stop=True,
                        skip_group_check=True,
                    )
                    # contiguous cast: psum (t,j) -> u (t,j)
                    eng = cp_engines[ncopy % 2]
                    ncopy += 1
                    dst = u[:, g * 512 : (g + 1) * 512]
                    if eng is nc.scalar:
                        eng.copy(dst, ups[:, :])
                    else:
                        eng.tensor_copy(dst, ups[:, :])
                # s0 partial: one accumulation group across all tiles
                nc.tensor.matmul(
                    s0p[:, :],
                    lhsT=xpl[:, p * 32 : (p + 1) * 32],
                    rhs=wch[:, qi * 512 : (qi + 1) * 512],
                    start=(p == 0),
                    stop=(p == NWT - 1),
                    skip_group_check=True,
                )
        s0 = sp.tile([32, 512], F32, tag="s_sb")
        nc.scalar.mul(s0[:, :], s0p[:, :], 1.0 / J)

        vexp = _allreduce_squash(
            tc, dp, sp, prp, scp, vp, repl, s0, epsb, r=0, single=single,
            final=False, dbg_out=outv,
        )

        # ---- routing iterations
        # Logits are recomputed from scratch each iteration against the
        # cumulative v (b^(2) = sum_t u*(v0+v1)), so no b_ij state is kept:
        # exp reads the PE-produced logits straight from PSUM.
        vexp0 = vexp
        for r in (1, 2):
            if r == 2:
                vsum = vp.tile([128, 512], BF16, tag="vsum")
                nc.vector.tensor_add(vsum[:, :], vexp0[:, :], vexp[:, :])
                vcur = vsum
            else:
                vcur = vexp
            sps = spp.tile([32, 512], F32, tag="s")
            ds, softs = [], {}

            def emit_soft(k, ds=ds, softs=softs):
                # exp of the PSUM logits (one ACT op), then z = sum_j on DVE
                cte = ep.tile([128, 256], BF16, tag=f"cte{k}", name=f"cte{k}")
                nc.scalar.activation(cte[:, :], ds[k][:, :], ExpF)
                z = ep.tile([128, 8], F32, tag=f"z{k}", name=f"z{k}")
                nc.vector.tensor_reduce(
                    z[:, :],
                    cte[:, :].rearrange("p (g j) -> p g j", g=8),
                    mybir.AxisListType.X,
                    mybir.AluOpType.add,
                )
                softs[k] = (cte, z)

            for k in range(NCH):
                usl = u[:, k * 4096 : (k + 1) * 4096].rearrange(
                    "p (g t j) -> p g t j", g=8, t=16
                )
                # q = u * v_cum, written in (t, g, j) layout
                q = qp.tile([128, 4096], BF16, tag="q")
                vb = (
                    vcur[:, :]
                    .rearrange("p (t j) -> p t j", t=16)
                    .unsqueeze(1)
                    .to_broadcast([128, 8, 16, 32])
                )
                nc.vector.tensor_mul(
                    q[:, :].rearrange("p (t g j) -> p g t j", t=16, g=8), usl, vb
                )
                # t-sum on PE: 16 accumulating matmuls with identity weights
                dps = dpp.tile([128, 256], F32, tag="d")
                for tt in range(16):
                    nc.tensor.matmul(
                        dps[:, :],
                        lhsT=ident[:, :],
                        rhs=q[:, tt * 256 : (tt + 1) * 256],
                        start=(tt == 0),
                        stop=(tt == 15),
                        skip_group_check=True,
                    )
                ds.append(dps)
                if k >= 1:
                    emit_soft(k - 1)
            emit_soft(NCH - 1)

            invzs = []
            for k in range(NCH):
                invz = ep.tile([128, 8], F32, tag=f"iz{k}", name=f"iz{k}")
                nc.vector.reciprocal(invz[:, :], softs[k][1][:, :])
                invzs.append(invz)
            # c = e/z on ACT (per-partition scale), freeing DVE for q/p2
            ccs = []
            for k in range(NCH):
                cc = ep.tile([128, 256], BF16, tag=f"cc{k}", name=f"cc{k}")
                for g in range(8):
                    nc.scalar.activation(
                        cc[:, g * 32 : (g + 1) * 32],
                        softs[k][0][:, g * 32 : (g + 1) * 32],
                        mybir.ActivationFunctionType.Identity,
                        scale=invzs[k][:, g : g + 1],
                    )
                ccs.append(cc)

            for k in range(NCH):
                usl = u[:, k * 4096 : (k + 1) * 4096].rearrange(
                    "p (g t j) -> p g t j", g=8, t=16
                )
                p2 = pp.tile([128, 4096], BF16, tag="p2")
                ccb = (
                    ccs[k][:, :]
                    .rearrange("p (g j) -> p g j", g=8)
                    .unsqueeze(2)
                    .to_broadcast([128, 8, 16, 32])
                )
                nc.vector.tensor_mul(
                    p2[:, :].rearrange("p (g t j) -> p g t j", g=8, t=16), usl, ccb
                )
                for gl in range(8):
                    nc.tensor.matmul(
                        sps[:, :],
                        lhsT=ones[:, :],
                        rhs=p2[:, gl * 512 : (gl + 1) * 512],
                        start=(k == 0 and gl == 0),
                        stop=(k == NCH - 1 and gl == 7),
                        skip_group_check=True,
                    )
            ssb = sp.tile([32, 512], F32, tag="s_sb")
            nc.scalar.copy(ssb[:, :], sps[:, :])

            vexp = _allreduce_squash(
                tc, dp, sp, prp, scp, vp, repl, ssb, epsb, r=r, single=single,
                final=(r == 2), dbg_out=outv,
            )

        nc.sync.dma_start(outv[0:4, :], vexp[:, :])


def _allreduce_squash(tc, dp, sp, prp, scp, vp, repl, s_part, epsb, r,
                      single=False, final=False, dbg_out=None):
    """AllReduce s [32,512] across cores, then v = squash(s).

    final=False: AllReduce; returns v replicated to 128 partitions as bf16.
    final=True: ReduceScatter; each core squashes its 4 batch rows and
    returns v_shard [4,512] f32 (host concatenates the 8 shards).
    squash via v = s * sqrt(ssq+eps) / (1+ssq)  (rsqrt through exp/ln so
    only the natural_log_exp ACT table set is ever needed).
    """
    nc = tc.nc
    P = 4 if final else 32
    ccin = dp.tile([32, 512], F32, tag=f"ccin{r}")
    ccout = dp.tile([P, 512], F32, tag=f"ccout{r}")
    nc.sync.dma_start(ccin[:, :], s_part[:, :])
    if single:
        nc.sync.dma_start(ccout[:, :], ccin[0:P, :])
    elif final:
        nc.gpsimd.collective_compute(
            "ReduceScatter",
            mybir.AluOpType.add,
            replica_groups=[list(range(NCORES))],
            ins=[ccin[:, :].opt()],
            outs=[ccout[:, :].opt()],
        )
    else:
        nc.gpsimd.collective_compute(
            "AllReduce",
            mybir.AluOpType.add,
            replica_groups=[list(range(NCORES))],
            ins=[ccin[:, :].opt()],
            outs=[ccout[:, :].opt()],
        )
    s = sp.tile([P, 512], F32, tag="s_full" if not final else "s_fin")
    nc.sync.dma_start(s[:, :], ccout[:, :])
    if os.environ.get("DBG_S") == str(r) and not final:
        nc.sync.dma_start(dbg_out, s[:, :])

    if not final:
        # replicate raw s to 128 partitions on PE early; the squash scale is
        # applied afterwards on the replicated copy (hides the PE latency
        # under the scalar squash chain)
        rps = prp.tile([128, 512], F32, tag="repl")
        nc.tensor.matmul(
            rps[:, :], lhsT=repl[:, :], rhs=s[:, :], start=True, stop=True,
            skip_group_check=True,
        )

    sq = sp.tile([P, 512], F32, tag="sq" if not final else "sqf")
    nc.scalar.square(sq[:, :], s[:, :])
    ssq = sp.tile([P, 32], F32, tag="ssq" if not final else "ssqf")
    nc.vector.tensor_reduce(
        ssq[:, :],
        sq[:, :].rearrange("p (t j) -> p j t", t=16),
        mybir.AxisListType.X,
        mybir.AluOpType.add,
    )
    t1 = sp.tile([P, 32], F32, tag="t1" if not final else "t1f")
    nc.vector.tensor_scalar_add(t1[:, :], ssq[:, :], 1.0)
    it1 = sp.tile([P, 32], F32, tag="it1" if not final else "it1f")
    nc.vector.reciprocal(it1[:, :], t1[:, :])
    lnr = sp.tile([P, 32], F32, tag="lnr" if not final else "lnrf")
    nc.scalar.activation(lnr[:, :], ssq[:, :], LnF, bias=epsb[0:P, :])
    w = sp.tile([P, 32], F32, tag="wrt" if not final else "wrtf")
    nc.scalar.activation(w[:, :], lnr[:, :], ExpF, scale=0.5)
    sc = sp.tile([P, 32], F32, tag="sc" if not final else "scf")
    nc.vector.tensor_mul(sc[:, :], w[:, :], it1[:, :])

    if final:
        scb = sc[:, :].unsqueeze(1).to_broadcast([P, 16, 32])
        v = sp.tile([P, 512], F32, tag=f"v{r}")
        nc.vector.tensor_mul(
            v[:, :].rearrange("p (t j) -> p t j", t=16),
            s[:, :].rearrange("p (t j) -> p t j", t=16),
            scb,
        )
        return v

    # replicate the scale to 128 partitions via PE, then scale rps
    scps = scp.tile([128, 32], F32, tag="scps")
    nc.tensor.matmul(
        scps[:, :], lhsT=repl[:, :], rhs=sc[:, :], start=True, stop=True,
        skip_group_check=True,
    )
    scf = sp.tile([128, 32], F32, tag="scf")
    nc.scalar.copy(scf[:, :], scps[:, :])
    vexp = vp.tile([128, 512], BF16, tag="vexp")
    nc.vector.tensor_mul(
        vexp[:, :].rearrange("p (t j) -> p t j", t=16),
        rps[:, :].rearrange("p (t j) -> p t j", t=16),
        scf[:, :].unsqueeze(1).to_broadcast([128, 16, 32]),
    )
    return vexp


@functools.lru_cache(maxsize=2)
def _get_nc(single=False):
    return _build_program(single=single)


def _prep_inputs(inputs, W):
    """Build per-core input maps (host-side layout only)."""
    inputs = np.asarray(inputs, dtype=np.float32)
    W = np.asarray(W, dtype=np.float32)
    W0 = W[0]  # [J, I, T, D]

    # delta_b ones [K=(i_sub 4, b 32), M=(b' 32)]
    ones = np.zeros((4, 32, 32), dtype=np.float32)
    for b in range(32):
        ones[:, b, b] = 1.0
    ones = ones.reshape(128, 32).astype(ml_dtypes.bfloat16)

    # v replication matrix [K=b 32, M=(k4, b' 32)=128]
    repl = np.zeros((32, 4, 32), dtype=np.float32)
    for b in range(32):
        repl[b, :, b] = 1.0
    repl = repl.reshape(32, 128)

    ident = np.eye(128, dtype=np.float32).astype(ml_dtypes.bfloat16)

    in_maps = []
    for c in range(NCORES):
        isl = slice(c * ILOC, (c + 1) * ILOC)
        ws = W0[:, isl]  # [J, 256, T, D]
        # wt[p, (gl, i_sub, d), (t, j)] ; i = (2p+gl)*4 + i_sub
        A = ws.transpose(1, 3, 2, 0)  # [i, d, t, j]
        A = A.reshape(NWT, 128, T * J)  # rows=(gl2,i_sub4,d16)
        # chunk-major pack: [NCHD, 128, WCH*512], contiguous per partition
        wtc = np.ascontiguousarray(
            A.reshape(NCHD, WCH, 128, 512).transpose(0, 2, 1, 3).reshape(
                NCHD, 128, WCH * 512
            )
        ).astype(ml_dtypes.bfloat16)

        xs = inputs[:, isl]  # [b, 256, d]
        xt = xs.transpose(1, 2, 0)  # [i, d, b]
        xplc = xt.reshape(NWT, 128, B)  # per-tile plain lhsT
        # pack to [128, NWT*32] (tile-major along free)
        xplc = np.ascontiguousarray(
            xplc.transpose(1, 0, 2).reshape(128, NWT * B)
        ).astype(ml_dtypes.bfloat16)

        xt4 = xt.reshape(NWT, 2, 4, D, B)  # p, gl, i_sub, d, b
        xdc = np.zeros((NWT, 2, 4, D, 4, B), dtype=np.float32)
        ar = np.arange(4)
        # advanced indexing: result axes [i_sub, p, gl, d, b]
        xdc[:, :, ar, :, ar, :] = xt4.transpose(2, 0, 1, 3, 4)
        xdc = xdc.reshape(NWT, 128, 128)
        xdc = np.ascontiguousarray(
            xdc.transpose(1, 0, 2).reshape(128, NWT * 128)
        ).astype(ml_dtypes.bfloat16)

        in_maps.append(
            {
                "wt": wtc, "xd": xdc, "xplain": xplc, "ident": ident,
                "onesdb": ones, "repl": repl,
                "warm": np.zeros((1, 8), dtype=np.float32),
            }
        )
    return in_maps


def assemble(results):
    """Concatenate the 8 per-core ReduceScatter shards (4 batch rows each)
    into the full [B, J, T] output."""
    v = np.concatenate(
        [np.asarray(results[c]["outv"])[0:4] for c in range(NCORES)], axis=0
    )  # [32, 512] = [b, (t, j)]
    return np.ascontiguousarray(
        v.reshape(B, T, J).transpose(0, 2, 1)
    ).astype(np.float32)


def kernel(inputs, W):
    import concourse.bass_utils as bass_utils

    nc = _get_nc()
    in_maps = _prep_inputs(inputs, W)
    res = bass_utils.run_bass_kernel_spmd(nc, in_maps, list(range(NCORES)))
    return assemble(res.results)


# revision 32
# speedup vs baseline: 1.0495x; 1.0495x over previous
"""CapsuleLayer (B=32, J=32, I=2048, T=16, D=16, 3 routing iters) on 8 TRN2 cores.

Strategy: shard input-capsule axis I across the 8 cores (I_loc = 256).
W reads at the HBM roofline (bf16, 4.2 MB/core); all routing state except
the tiny s[b,j,t] (64 KB, AllReduce x3) is core-local.

Per core:
  - u_hat on TensorE: K=(i_sub 4, d 16)=64, M=(i_sub 4, b 32)=128,
    N=(t,j)=512, host-prepacked block-diagonal x weights (bf16) and W
    streamed as bf16 rhs.  W host layout puts free=(t,j) so the PSUM->SBUF
    u copy is a contiguous cast (2x mode), alternating ACT/DVE.
  - s0 partial fused into the same W pass (xplain lhsT), single PSUM
    accumulation group across all tiles (skip_group_check).
  - routing: q = u*v on DVE (bf16 2x) into (t,g,j) layout; the t-sum is
    done on TensorE (identity-weight restream, 16 accumulating matmuls
    into one PSUM bank per chunk); softmax exp on ACT with accum_out
    giving z for free; p2 = u*c on DVE; i-sum via ones-delta restream.
  - squash uses only Exp/Ln ACT functions (rsqrt via exp(0.5*ln)), so a
    single ACT table set (natural_log_exp) is loaded once - no set
    thrashing on the critical path.
  - a dummy AllReduce at t=0 warms up the collective mesh so the first
    real AllReduce only pays steady-state latency.
"""

import functools
import os
import sys

import numpy as np

sys.path.insert(0, "/opt/trn_rl_repo")

import ml_dtypes  # noqa: E402

import concourse.bass as bass  # noqa: E402
import concourse.bacc as bacc  # noqa: E402
import concourse.mybir as mybir  # noqa: E402
import concourse.tile as tile  # noqa: E402

F32 = mybir.dt.float32
BF16 = mybir.dt.bfloat16

# Make the ACT table-load pass resolve Exp and Ln to the one set that has
# both (natural_log_exp_and_others) instead of greedily alternating between
# exp_and_others and natural_log (a ~1.3us table load per switch, on the
# critical path).  Set indices are preserved; we only hide Exp/Ln from the
# other sets so the chooser can't pick them.
_orig_get_act_tables = bacc.get_activation_tables


def _patched_get_act_tables(arch):
    tables = _orig_get_act_tables(arch)
    combined = None
    for name, fns in tables.items():
        if (mybir.ActivationFunctionType.Exp in fns
                and mybir.ActivationFunctionType.Ln in fns):
            combined = name
            break
    if combined is not None:
        for name, fns in tables.items():
            if name != combined:
                fns.discard(mybir.ActivationFunctionType.Exp)
                fns.discard(mybir.ActivationFunctionType.Ln)
    return tables


bacc.get_activation_tables = _patched_get_act_tables

NCORES = 8
B, J, I, T, D = 32, 32, 2048, 16, 16
ILOC = I // NCORES          # 256
G = ILOC // 4               # 64 i-groups of 4
EPS = 1e-9

NWT = G // 2                # 32 w dram tiles, each [128, 512] covers 2 g
WCH = 8                     # w tiles per DMA chunk
NCHD = NWT // WCH           # 4 dma chunks
NCH = 8                     # routing chunks (8 g per chunk)

ExpF = mybir.ActivationFunctionType.Exp
LnF = mybir.ActivationFunctionType.Ln


def _build_program(single=False):
    nc = bacc.Bacc(
        "TRN2",
        target_bir_lowering=False,
        debug=False,
        enable_asserts=False,
        num_devices=1 if single else NCORES,
    )

    wt_d = nc.dram_tensor("wt", [NCHD, 128, WCH * 512], BF16, kind="ExternalInput")
    xd_d = nc.dram_tensor("xd", [128, NWT * 128], BF16, kind="ExternalInput")
    xp_d = nc.dram_tensor("xplain", [128, NWT * 32], BF16, kind="ExternalInput")
    id_d = nc.dram_tensor("ident", [128, 128], BF16, kind="ExternalInput")
    warm_d = nc.dram_tensor("warm", [1, 8], F32, kind="ExternalInput")
    ones_d = nc.dram_tensor("onesdb", [128, 32], BF16, kind="ExternalInput")
    repl_d = nc.dram_tensor("repl", [32, 128], BF16, kind="ExternalInput")
    out_d = nc.dram_tensor("outv", [32, 512], F32, kind="ExternalOutput")

    with tile.TileContext(nc) as tc:
        _capsule(
            tc, wt_d.ap(), xd_d.ap(), xp_d.ap(), id_d.ap(), ones_d.ap(),
            repl_d.ap(), out_d.ap(), warm_d.ap(), single=single,
        )
    nc.compile()
    return nc


def _capsule(tc, wt, xd, xpd, identd, ones_dram, repl_dram, outv, warm_dram,
             single=False):
    nc = tc.nc
    from contextlib import ExitStack

    ctx = ExitStack()
    with ctx:
        up = ctx.enter_context(tc.tile_pool(name="u", bufs=1))
        wp = ctx.enter_context(tc.tile_pool(name="w", bufs=2))
        xp = ctx.enter_context(tc.tile_pool(name="x", bufs=1))
        cp = ctx.enter_context(tc.tile_pool(name="consts", bufs=1))
        qp = ctx.enter_context(tc.tile_pool(name="q", bufs=3))
        pp = ctx.enter_context(tc.tile_pool(name="p2", bufs=2))
        ep = ctx.enter_context(tc.tile_pool(name="soft", bufs=1))
        sp = ctx.enter_context(tc.tile_pool(name="small", bufs=2))
        vp = ctx.enter_context(tc.tile_pool(name="vexp", bufs=2))
        pup = ctx.enter_context(tc.tile_pool(name="upsum", bufs=3, space="PSUM"))
        ps0 = ctx.enter_context(tc.tile_pool(name="s0psum", bufs=1, space="PSUM"))
        dpp = ctx.enter_context(tc.tile_pool(name="dpsum", bufs=2, space="PSUM"))
        spp = ctx.enter_context(tc.tile_pool(name="spsum", bufs=1, space="PSUM"))
        prp = ctx.enter_context(tc.tile_pool(name="rpsum", bufs=1, space="PSUM"))
        scp = None
        dp = ctx.enter_context(tc.tile_pool(name="dram", bufs=8, space="DRAM"))

        # ---- collective warm-up: a dummy AllReduce rendezvous absorbs the
        # per-core ncfw boot/setup variance while phase A's DMAs run.
        if not single:
            wcc_in = dp.tile([1, 8], F32, tag="wccin", name="wcc_in")
            wcc = dp.tile([1, 8], F32, tag="wcc", name="wcc_out")
            nc.sync.dma_start(wcc_in[:, :], warm_dram)
            nc.gpsimd.collective_compute(
                "AllReduce",
                mybir.AluOpType.add,
                replica_groups=[list(range(NCORES))],
                ins=[wcc_in[:, :].opt()],
                outs=[wcc[:, :].opt()],
            )

        # ---- ACT table warm-up: Exp+Ln only => single natural_log_exp set
        wact = cp.tile([1, 8], F32, name="warm_act")
        wsb = cp.tile([1, 8], F32, name="warm_sb")
        nc.vector.memset(wsb[:, :], 1.0)
        nc.scalar.activation(wact[:, :], wsb[:, :], ExpF)
        nc.scalar.activation(wact[:, :], wsb[:, :], LnF)
        epsb = cp.tile([32, 1], F32, name="eps_bias")
        nc.vector.memset(epsb[:, :], EPS)

        # ---- persistent tiles
        u = up.tile([128, G * 512], BF16)          # [(i_sub,b), (g,t,j)]
        xall = xp.tile([128, NWT * 128], BF16)     # block-diag x lhsT
        xpl = xp.tile([128, NWT * 32], BF16)       # plain x lhsT
        ident = cp.tile([128, 128], BF16)          # identity for t-restream
        ones = cp.tile([128, 32], BF16)            # b-delta for i-restream
        repl = cp.tile([32, 128], BF16)            # v replication matrix

        nc.sync.dma_start(xall[:, :], xd)
        nc.sync.dma_start(xpl[:, :], xpd)
        nc.sync.dma_start(ident[:, :], identd)
        nc.sync.dma_start(ones[:, :], ones_dram)
        nc.sync.dma_start(repl[:, :], repl_dram)

        # ---- phase A: u_hat + fused s0 partial
        cp_engines = [nc.scalar, nc.vector]
        s0p = ps0.tile([32, 512], F32, tag="s0")
        ncopy = 0
        for c in range(NCHD):
            wch = wp.tile([128, WCH * 512], BF16, tag="w")
            nc.sync.dma_start(wch[:, :], wt[c])
            for qi in range(WCH):
                p = c * WCH + qi
                for gl in range(2):
                    g = 2 * p + gl
                    ups = pup.tile([128, 512], F32)
                    nc.tensor.matmul(
                        ups[:, :],
                        lhsT=xall[gl * 64 : (gl + 1) * 64, p * 128 : (p + 1) * 128],
                        rhs=wch[gl * 64 : (gl + 1) * 64, qi * 512 : (qi + 1) * 512],
                        start=True,
                        stop=True,
                        skip_group_check=True,
                    )
                    # contiguous cast: psum (t,j) -> u (t,j)
                    eng = cp_engines[ncopy % 2]
                    ncopy += 1
                    dst = u[:, g * 512 : (g + 1) * 512]
                    if eng is nc.scalar:
                        eng.copy(dst, ups[:, :])
                    else:
                        eng.tensor_copy(dst, ups[:, :])
                # s0 partial: one accumulation group across all tiles
                nc.tensor.matmul(
                    s0p[:, :],
                    lhsT=xpl[:, p * 32 : (p + 1) * 32],
                    rhs=wch[:, qi * 512 : (qi + 1) * 512],
                    start=(p == 0),
                    stop=(p == NWT - 1),
                    skip_group_check=True,
                )
        s0 = sp.tile([32, 512], F32, tag="s_sb")
        nc.scalar.mul(s0[:, :], s0p[:, :], 1.0 / J)

        vexp = _allreduce_squash(
            tc, dp, sp, prp, scp, vp, repl, s0, epsb, r=0, single=single,
            final=False, dbg_out=outv,
        )

        # ---- routing iterations
        # Logits are recomputed from scratch each iteration against the
        # cumulative v (b^(2) = sum_t u*(v0+v1)), so no b_ij state is kept:
        # exp reads the PE-produced logits straight from PSUM.
        vexp0 = vexp
        for r in (1, 2):
            if r == 2:
                vsum = vp.tile([128, 512], BF16, tag="vsum")
                nc.vector.tensor_add(vsum[:, :], vexp0[:, :], vexp[:, :])
                vcur = vsum
            else:
                vcur = vexp
            sps = spp.tile([32, 512], F32, tag="s")
            ds, ctes = [], []

            for k in range(NCH):
                usl = u[:, k * 4096 : (k + 1) * 4096].rearrange(
                    "p (g t j) -> p g t j", g=8, t=16
                )
                # q = u * v_cum, written in (t, g, j) layout
                q = qp.tile([128, 4096], BF16, tag="q")
                vb = (
                    vcur[:, :]
                    .rearrange("p (t j) -> p t j", t=16)
                    .unsqueeze(1)
                    .to_broadcast([128, 8, 16, 32])
                )
                nc.vector.tensor_mul(
                    q[:, :].rearrange("p (t g j) -> p g t j", t=16, g=8), usl, vb
                )
                # t-sum on PE: 16 accumulating matmuls with identity weights
                dps = dpp.tile([128, 256], F32, tag="d")
                for tt in range(16):
                    nc.tensor.matmul(
                        dps[:, :],
                        lhsT=ident[:, :],
                        rhs=q[:, tt * 256 : (tt + 1) * 256],
                        start=(tt == 0),
                        stop=(tt == 15),
                        skip_group_check=True,
                    )
                ds.append(dps)
                if k >= 1:
                    # exp of the PSUM logits (one ACT op per chunk)
                    cte = ep.tile([128, 256], BF16, tag=f"cte{k-1}",
                                  name=f"cte{k-1}")
                    nc.scalar.activation(cte[:, :], ds[k - 1][:, :], ExpF)
                    ctes.append(cte)
            cte = ep.tile([128, 256], BF16, tag=f"cte{NCH-1}", name=f"cte{NCH-1}")
            nc.scalar.activation(cte[:, :], ds[NCH - 1][:, :], ExpF)
            ctes.append(cte)

            # softmax denominators, reciprocals and c = e/z, all on DVE in
            # straight blocks (no cross-engine latency in the steady state)
            zs, invzs, ccs = [], [], []
            for k in range(NCH):
                z = ep.tile([128, 8], F32, tag=f"z{k}", name=f"z{k}")
                nc.vector.tensor_reduce(
                    z[:, :],
                    ctes[k][:, :].rearrange("p (g j) -> p g j", g=8),
                    mybir.AxisListType.X,
                    mybir.AluOpType.add,
                )
                zs.append(z)
            for k in range(NCH):
                invz = ep.tile([128, 8], F32, tag=f"iz{k}", name=f"iz{k}")
                nc.vector.reciprocal(invz[:, :], zs[k][:, :])
                invzs.append(invz)
            for k in range(NCH):
                cc = ep.tile([128, 256], BF16, tag=f"cc{k}", name=f"cc{k}")
                nc.vector.tensor_mul(
                    cc[:, :].rearrange("p (g j) -> p g j", g=8),
                    ctes[k][:, :].rearrange("p (g j) -> p g j", g=8),
                    invzs[k][:, :].unsqueeze(2).to_broadcast([128, 8, 32]),
                )
                ccs.append(cc)

            for k in range(NCH):
                usl = u[:, k * 4096 : (k + 1) * 4096].rearrange(
                    "p (g t j) -> p g t j", g=8, t=16
                )
                p2 = pp.tile([128, 4096], BF16, tag="p2")
                ccb = (
                    ccs[k][:, :]
                    .rearrange("p (g j) -> p g j", g=8)
                    .unsqueeze(2)
                    .to_broadcast([128, 8, 16, 32])
                )
                nc.vector.tensor_mul(
                    p2[:, :].rearrange("p (g t j) -> p g t j", g=8, t=16), usl, ccb
                )
                for gl in range(8):
                    nc.tensor.matmul(
                        sps[:, :],
                        lhsT=ones[:, :],
                        rhs=p2[:, gl * 512 : (gl + 1) * 512],
                        start=(k == 0 and gl == 0),
                        stop=(k == NCH - 1 and gl == 7),
                        skip_group_check=True,
                    )
            ssb = sp.tile([32, 512], F32, tag="s_sb")
            nc.scalar.copy(ssb[:, :], sps[:, :])

            vexp = _allreduce_squash(
                tc, dp, sp, prp, scp, vp, repl, ssb, epsb, r=r, single=single,
                final=(r == 2), dbg_out=outv,
            )

        nc.sync.dma_start(outv[0:4, :], vexp[:, :])


def _allreduce_squash(tc, dp, sp, prp, scp, vp, repl, s_part, epsb, r,
                      single=False, final=False, dbg_out=None):
    """AllReduce s [32,512] across cores, then v = squash(s).

    final=False: AllReduce; returns v replicated to 128 partitions as bf16.
    final=True: ReduceScatter; each core squashes its 4 batch rows and
    returns v_shard [4,512] f32 (host concatenates the 8 shards).
    squash via v = s * sqrt(ssq+eps) / (1+ssq)  (rsqrt through exp/ln so
    only the natural_log_exp ACT table set is ever needed).
    """
    nc = tc.nc
    P = 4 if final else 32
    ccin = dp.tile([32, 512], F32, tag=f"ccin{r}")
    ccout = dp.tile([P, 512], F32, tag=f"ccout{r}")
    nc.sync.dma_start(ccin[:, :], s_part[:, :])
    if single:
        nc.sync.dma_start(ccout[:, :], ccin[0:P, :])
    elif final:
        nc.gpsimd.collective_compute(
            "ReduceScatter",
            mybir.AluOpType.add,
            replica_groups=[list(range(NCORES))],
            ins=[ccin[:, :].opt()],
            outs=[ccout[:, :].opt()],
        )
    else:
        nc.gpsimd.collective_compute(
            "AllReduce",
            mybir.AluOpType.add,
            replica_groups=[list(range(NCORES))],
            ins=[ccin[:, :].opt()],
            outs=[ccout[:, :].opt()],
        )
    s = sp.tile([P, 512], F32, tag="s_full" if not final else "s_fin")
    nc.sync.dma_start(s[:, :], ccout[:, :])
    if os.environ.get("DBG_S") == str(r) and not final:
        nc.sync.dma_start(dbg_out, s[:, :])

    sq = sp.tile([P, 512], F32, tag="sq" if not final else "sqf")
    nc.scalar.square(sq[:, :], s[:, :])
    ssq = sp.tile([P, 32], F32, tag="ssq" if not final else "ssqf")
    nc.vector.tensor_reduce(
        ssq[:, :],
        sq[:, :].rearrange("p (t j) -> p j t", t=16),
        mybir.AxisListType.X,
        mybir.AluOpType.add,
    )
    t1 = sp.tile([P, 32], F32, tag="t1" if not final else "t1f")
    nc.vector.tensor_scalar_add(t1[:, :], ssq[:, :], 1.0)
    it1 = sp.tile([P, 32], F32, tag="it1" if not final else "it1f")
    nc.vector.reciprocal(it1[:, :], t1[:, :])
    lnr = sp.tile([P, 32], F32, tag="lnr" if not final else "lnrf")
    nc.scalar.activation(lnr[:, :], ssq[:, :], LnF, bias=epsb[0:P, :])
    w = sp.tile([P, 32], F32, tag="wrt" if not final else "wrtf")
    nc.scalar.activation(w[:, :], lnr[:, :], ExpF, scale=0.5)
    sc = sp.tile([P, 32], F32, tag="sc" if not final else "scf")
    nc.vector.tensor_mul(sc[:, :], w[:, :], it1[:, :])

    scb = sc[:, :].unsqueeze(1).to_broadcast([P, 16, 32])
    if final:
        v = sp.tile([P, 512], F32, tag=f"v{r}")
        nc.vector.tensor_mul(
            v[:, :].rearrange("p (t j) -> p t j", t=16),
            s[:, :].rearrange("p (t j) -> p t j", t=16),
            scb,
        )
        return v

    vbf = sp.tile([32, 512], BF16, tag=f"vbf{r}")
    nc.vector.tensor_mul(
        vbf[:, :].rearrange("p (t j) -> p t j", t=16),
        s[:, :].rearrange("p (t j) -> p t j", t=16),
        scb,
    )
    # replicate to 128 partitions via PE: repl.T @ vbf
    rps = prp.tile([128, 512], F32, tag="repl")
    nc.tensor.matmul(
        rps[:, :], lhsT=repl[:, :], rhs=vbf[:, :], start=True, stop=True,
        skip_group_check=True,
    )
    vexp = vp.tile([128, 512], BF16, tag="vexp")
    nc.scalar.copy(vexp[:, :], rps[:, :])
    return vexp


@functools.lru_cache(maxsize=2)
def _get_nc(single=False):
    return _build_program(single=single)


def _prep_inputs(inputs, W):
    """Build per-core input maps (host-side layout only)."""
    inputs = np.asarray(inputs, dtype=np.float32)
    W = np.asarray(W, dtype=np.float32)
    W0 = W[0]  # [J, I, T, D]

    # delta_b ones [K=(i_sub 4, b 32), M=(b' 32)]
    ones = np.zeros((4, 32, 32), dtype=np.float32)
    for b in range(32):
        ones[:, b, b] = 1.0
    ones = ones.reshape(128, 32).astype(ml_dtypes.bfloat16)

    # v replication matrix [K=b 32, M=(k4, b' 32)=128]
    repl = np.zeros((32, 4, 32), dtype=np.float32)
    for b in range(32):
        repl[b, :, b] = 1.0
    repl = repl.reshape(32, 128).astype(ml_dtypes.bfloat16)

    ident = np.eye(128, dtype=np.float32).astype(ml_dtypes.bfloat16)

    in_maps = []
    for c in range(NCORES):
        isl = slice(c * ILOC, (c + 1) * ILOC)
        ws = W0[:, isl]  # [J, 256, T, D]
        # wt[p, (gl, i_sub, d), (t, j)] ; i = (2p+gl)*4 + i_sub
        A = ws.transpose(1, 3, 2, 0)  # [i, d, t, j]
        A = A.reshape(NWT, 128, T * J)  # rows=(gl2,i_sub4,d16)
        # chunk-major pack: [NCHD, 128, WCH*512], contiguous per partition
        wtc = np.ascontiguousarray(
            A.reshape(NCHD, WCH, 128, 512).transpose(0, 2, 1, 3).reshape(
                NCHD, 128, WCH * 512
            )
        ).astype(ml_dtypes.bfloat16)

        xs = inputs[:, isl]  # [b, 256, d]
        xt = xs.transpose(1, 2, 0)  # [i, d, b]
        xplc = xt.reshape(NWT, 128, B)  # per-tile plain lhsT
        # pack to [128, NWT*32] (tile-major along free)
        xplc = np.ascontiguousarray(
            xplc.transpose(1, 0, 2).reshape(128, NWT * B)
        ).astype(ml_dtypes.bfloat16)

        xt4 = xt.reshape(NWT, 2, 4, D, B)  # p, gl, i_sub, d, b
        xdc = np.zeros((NWT, 2, 4, D, 4, B), dtype=np.float32)
        ar = np.arange(4)
        # advanced indexing: result axes [i_sub, p, gl, d, b]
        xdc[:, :, ar, :, ar, :] = xt4.transpose(2, 0, 1, 3, 4)
        xdc = xdc.reshape(NWT, 128, 128)
        xdc = np.ascontiguousarray(
            xdc.transpose(1, 0, 2).reshape(128, NWT * 128)
        ).astype(ml_dtypes.bfloat16)

        in_maps.append(
            {
                "wt": wtc, "xd": xdc, "xplain": xplc, "ident": ident,
                "onesdb": ones, "repl": repl,
                "warm": np.zeros((1, 8), dtype=np.float32),
            }
        )
    return in_maps


def assemble(results):
    """Concatenate the 8 per-core ReduceScatter shards (4 batch rows each)
    into the full [B, J, T] output."""
    v = np.concatenate(
        [np.asarray(results[c]["outv"])[0:4] for c in range(NCORES)], axis=0
    )  # [32, 512] = [b, (t, j)]
    return np.ascontiguousarray(
        v.reshape(B, T, J).transpose(0, 2, 1)
    ).astype(np.float32)


def kernel(inputs, W):
    import concourse.bass_utils as bass_utils

    nc = _get_nc()
    in_maps = _prep_inputs(inputs, W)
    res = bass_utils.run_bass_kernel_spmd(nc, in_maps, list(range(NCORES)))
    return assemble(res.results)
